# revision 1
# baseline (speedup 1.0000x reference)
"""Trainium2 Bass kernel for nn_Decoder: Linear + 4-layer GRU + tanh.

Model (fp32): z[256,128] -> Linear(W_lin[131072,128]) -> x[256,512,256]
  -> 4 stacked GRU layers (T=512, H=256, PyTorch gate order r,z,n)
  -> tanh -> out[256,256,512]

Sharding: pure data parallel over batch, 32 samples per core on 8 cores.

Per-core design ("option A", all matmul dsts at PSUM partition 0):
  - Everything lives transposed: h, x and the gate pre-activations are kept
    as [feature-dim on partitions, batch on free].  The recurrent matmul
    keeps the weights stationary (lhsT = Whh^T chunk [K=128 h-half,
    M=128 gate-chunk]) and streams h [128, 32].  Gate elementwise math runs
    directly in this layout, and h' feeds the next step's matmul with no
    transpose at all.
  - The 4 GRU layers run software-pipelined with skew 8 (layer l processes
    t = u - 8l at global step u).  Layers stack along PSUM *columns*
    (PG[128, l*192 + 32j + b]), so one strided-AP elementwise op serves all
    four layers; total free-size stays 256 per op.
  - Input projections x@Wih^T are batched 4 steps at a time (moving operand
    [128, 128]) into PGX, then copied to an SBUF ring indexed by the global
    step at which each column is consumed - off the critical path.
  - Biases enter PSUM via K=6 one-hot selector matmuls.
  - All matmuls bf16 (measured end-to-end max-abs-err ~8.5e-3 vs fp32).
  - Phase 0 (Linear) streams W_lin^T (bf16, 33 MB) and emits x directly in
    [h, t*32+b] layout via N=32 matmuls; b_lin via a K=16 selector matmul.
  - Output is written in [h, b] layout to DRAM; the host transposes.
"""

import sys

sys.path.insert(0, "/opt/trn_rl_repo")

import numpy as np

import concourse.bass as bass
import concourse.tile as tile
from concourse import bacc, mybir
from concourse.bass_utils import run_bass_kernel_spmd

F32 = mybir.dt.float32
BF16 = mybir.dt.bfloat16

N_CORES = 8
B = 32          # batch per core
H = 256         # GRU hidden size
HH = 128
G3 = 768
Z = 128
L = 4
SKEW = 8
RING = 8
T_FULL = 512
FEAT = T_FULL * H


def build_program(T=T_FULL, num_devices=N_CORES, phase0_lead=14, u_cap=None):
    U = T + (L - 1) * SKEW     # last step index + 1
    n_grp = T // 8
    n_win = max(1, T // 64)
    win_t = T // n_win
    feat = T * H

    nc = bacc.Bacc("TRN2", target_bir_lowering=False, debug=False,
                   num_devices=num_devices)

    d = {}
    d["zt"] = nc.dram_tensor("zt", [Z, B], BF16, kind="ExternalInput").ap()
    d["wlin"] = nc.dram_tensor("wlin", [Z, feat], BF16, kind="ExternalInput").ap()
    d["blin"] = nc.dram_tensor("blin", [n_grp, 16, HH], BF16, kind="ExternalInput").ap()
    d["bsel"] = nc.dram_tensor("bsel", [16, 512], BF16, kind="ExternalInput").ap()
    d["whh"] = nc.dram_tensor("whh", [HH, L * 12 * HH], BF16, kind="ExternalInput").ap()
    d["wih"] = nc.dram_tensor("wih", [HH, L * 12 * HH], BF16, kind="ExternalInput").ap()
    d["bhhS"] = nc.dram_tensor("bhhS", [2, 12, HH], BF16, kind="ExternalInput").ap()
    d["selB"] = nc.dram_tensor("selB", [12, 512], BF16, kind="ExternalInput").ap()
    d["bih6"] = nc.dram_tensor("bih6", [6, L * HH], BF16, kind="ExternalInput").ap()
    d["selx"] = nc.dram_tensor("selx", [6, G3], BF16, kind="ExternalInput").ap()
    d["out"] = nc.dram_tensor("out", [HH, T * 2 * B], F32, kind="ExternalOutput").ap()

    if u_cap is not None:
        U = u_cap
    with tile.TileContext(nc) as tc:
        _build(tc, nc, T, U, n_grp, n_win, win_t, phase0_lead, d)

    nc.compile()
    return nc


def _build(tc, nc, T, U, n_grp, n_win, win_t, lead, d):
    from contextlib import ExitStack
    ctx = ExitStack()

    res = ctx.enter_context(tc.tile_pool(name="res", bufs=1))
    wlinp = ctx.enter_context(tc.tile_pool(name="wlinp", bufs=3))
    pgp = ctx.enter_context(tc.tile_pool(name="pgp", bufs=2, space="PSUM"))
    pgxp = ctx.enter_context(tc.tile_pool(name="pgxp", bufs=2, space="PSUM"))
    gate = ctx.enter_context(tc.tile_pool(name="gate", bufs=2))
    outp = ctx.enter_context(tc.tile_pool(name="outp", bufs=2))

    SIG = mybir.ActivationFunctionType.Sigmoid
    TANH = mybir.ActivationFunctionType.Tanh

    # ---- resident SBUF ----
    zt_sb = res.tile([Z, B], BF16, tag="zt")
    bsel_sb = res.tile([16, 512], BF16, tag="bsel")
    whh_sb = res.tile([HH, L * 12 * HH], BF16, tag="whh")
    wih_sb = res.tile([HH, L * 12 * HH], BF16, tag="wih")
    bhhS_sb = res.tile([12, 2 * HH], BF16, tag="bhhS")
    selB_sb = res.tile([12, 512], BF16, tag="selB")
    bih6_sb = res.tile([6, L * HH], BF16, tag="bih6")
    selx_sb = res.tile([6, G3], BF16, tag="selx")
    # layer-0 input x^T windows [h-half][win] : [128, win_t*32] bf16
    xw = [[res.tile([HH, win_t * B], BF16, tag=f"xw{q}_{w}", name=f"xw{q}_{w}")
           for w in range(n_win)] for q in range(2)]
    # h ring: col ((l*2+q)*8 + t%8)*32, content h_l[t] half q as [h, b]
    hR = res.tile([HH, L * 2 * RING * B], BF16, tag="hR")
    # gx ring: slot = consume-step u%8; col slot*768 + l*192 + 32j
    gxS = res.tile([HH, RING * L * 192], BF16, tag="gxS")

    for name, t_sb in [("zt", zt_sb), ("bsel", bsel_sb), ("whh", whh_sb),
                       ("wih", wih_sb), ("bih6", bih6_sb),
                       ("selB", selB_sb), ("selx", selx_sb)]:
        nc.sync.dma_start(t_sb[:], d[name][:])
    for k in range(2):
        nc.sync.dma_start(bhhS_sb[:, k * HH:(k + 1) * HH], d["bhhS"][k])
    nc.vector.memset(hR[:], 0.0)

    d_out = d["out"]

    # ---- phase 0: x^T = W_lin^T-chunks.T @ z^T, 8 t-values per group ----
    def emit_ph0_group(g):
        t0 = g * 8
        wl = wlinp.tile([Z, 8 * H], BF16, tag="wl")
        nc.sync.dma_start(wl[:], d["wlin"][:, t0 * H:(t0 + 8) * H])
        bl = wlinp.tile([16, HH], BF16, tag="bl")
        nc.sync.dma_start(bl[:], d["blin"][g])
        px = pgxp.tile([HH, 512], F32, tag="pgx")
        nc.tensor.matmul(px[:], bl[:], bsel_sb[:], start=True, stop=False,
                         skip_group_check=True)
        for q in range(2):
            for tp in range(8):
                nc.tensor.matmul(
                    px[:, q * 256 + tp * B: q * 256 + (tp + 1) * B],
                    wl[:, tp * H + q * HH: tp * H + (q + 1) * HH],
                    zt_sb[:],
                    start=False, stop=(q == 1 and tp == 7),
                    skip_group_check=True)
        w = t0 // win_t
        c0 = (t0 % win_t) * B
        nc.scalar.copy(xw[0][w][:, c0:c0 + 8 * B], px[:, 0:256])
        nc.vector.tensor_copy(xw[1][w][:, c0:c0 + 8 * B], px[:, 256:512])

    for g in range(min(lead, n_grp)):
        emit_ph0_group(g)
    next_grp = min(lead, n_grp)

    MM = dict(skip_group_check=True)

    def wslice(wsb, l, q, j):
        c = ((l * 2 + q) * 6 + j) * HH
        return wsb[:, c:c + HH]

    def hslot(l, q, slot, n=1):
        c = ((l * 2 + q) * RING + slot) * B
        return hR[:, c:c + n * B]

    # ---- main loop ----
    for u in range(-4, U):
        # phase-0 pacing
        if u >= 0 and u % 8 == 0 and next_grp < n_grp:
            emit_ph0_group(next_grp)
            next_grp += 1

        # -- x-side batched projections (4 steps), staggered by layer --
        for l in range(L):
            tb = u + 4 - 9 * l
            if tb < 0 or tb >= T or tb % 4 != 0:
                continue
            pgx = pgxp.tile([HH, G3], F32, tag="pgx")
            nc.tensor.matmul(pgx[:, 0:512], bih6_sb[:, l * HH:(l + 1) * HH],
                             selx_sb[:, 0:512], start=True, stop=False, **MM)
            nc.tensor.matmul(pgx[:, 512:G3], bih6_sb[:, l * HH:(l + 1) * HH],
                             selx_sb[:, 512:G3], start=True, stop=False, **MM)
            for q in range(2):
                if l == 0:
                    w = tb // win_t
                    src = xw[q][w][:, (tb % win_t) * B:(tb % win_t) * B + 4 * B]
                else:
                    src = hslot(l - 1, q, tb % RING, n=4)
                for j in range(6):
                    nc.tensor.matmul(pgx[:, j * HH:(j + 1) * HH],
                                     wslice(wih_sb, l, q, j), src,
                                     start=False, stop=(q == 1 and j == 5), **MM)
            # scatter-copy to gxS: consume slot base (tb + 8l) % 8 == tb % 8
            base = (tb + 8 * l) % RING
            src_ap = pgx[:].rearrange("p (j t b) -> p j t b", j=6, t=4)
            dst = gxS[:].rearrange("p (s l j b) -> p l s j b", s=RING, l=L, j=6)
            dst_ap = dst[:, l, base:base + 4].rearrange("p t j b -> p j t b")
            nc.scalar.copy(dst_ap, src_ap)

        if u < 0:
            continue

        la = max(0, -(-(u - (T - 1)) // SKEW))  # ceil((u-T+1)/8)
        lb = min(L - 1, u // SKEW)
        nl = lb - la + 1
        slot = u % RING
        pslot = (u - 1) % RING

        pg = pgp.tile([HH, L * 256], F32, tag="pg")

        # -- bias + h-side matmuls (z, r, n chunk order) --
        # one bank-wide start=True matmul per layer-pair: interleaving two
        # accumulation groups in one PSUM bank loses the first group's data
        for k in range(2):
            if 2 * k + 1 < la or 2 * k > lb:
                continue
            nc.tensor.matmul(pg[:, k * 512:(k + 1) * 512],
                             bhhS_sb[:, k * HH:(k + 1) * HH], selB_sb[:],
                             start=True, stop=False, **MM)
        for jgrp in ((2, 3), (0, 1), (4, 5)):      # z first, then r, then n
            for l in range(la, lb + 1):
                for j in jgrp:
                    for q in range(2):
                        nc.tensor.matmul(
                            pg[:, l * 256 + j * B: l * 256 + (j + 1) * B],
                            wslice(whh_sb, l, q, j),
                            hslot(l, q, pslot),
                            start=False, stop=(jgrp == (4, 5) and j == 5 and q == 1),
                            **MM)

        # -- gates: strided APs over active layers --
        pg4 = pg[:].rearrange("p (l j b) -> p l j b", l=L, j=8)[:, la:lb + 1]
        gx4 = gxS[:, slot * (L * 192):(slot + 1) * (L * 192)] \
            .rearrange("p (l j b) -> p l j b", l=L, j=6)[:, la:lb + 1]
        hp4 = hR[:].rearrange("p (l q s b) -> p l q s b", l=L, q=2, s=RING)[:, la:lb + 1]

        zpre = gate.tile([HH, L * 64], BF16, tag="zpre")
        z_t = gate.tile([HH, L * 64], BF16, tag="z_t")
        rpre = gate.tile([HH, L * 64], BF16, tag="rpre")
        r_t = gate.tile([HH, L * 64], BF16, tag="r_t")
        rhn = gate.tile([HH, L * 64], BF16, tag="rhn")
        npre = gate.tile([HH, L * 64], BF16, tag="npre")
        n_t = gate.tile([HH, L * 64], BF16, tag="n_t")
        omz = gate.tile([HH, L * 64], BF16, tag="omz")
        zh = gate.tile([HH, L * 64], BF16, tag="zh")
        nom = gate.tile([HH, L * 64], BF16, tag="nom")

        def g4(tl):
            return tl[:, la * 64:(lb + 1) * 64].rearrange(
                "p (l q b) -> p l q b", l=nl, q=2)

        # z path (off critical chain)
        nc.vector.tensor_add(g4(zpre), pg4[:, :, 2:4], gx4[:, :, 2:4])
        nc.scalar.activation(g4(z_t), g4(zpre), SIG)
        nc.gpsimd.tensor_scalar(out=g4(omz), in0=g4(z_t), scalar1=-1.0,
                                scalar2=1.0, op0=mybir.AluOpType.mult,
                                op1=mybir.AluOpType.add)
        nc.gpsimd.tensor_mul(g4(zh), g4(z_t), hp4[:, :, :, pslot])
        # r path
        nc.vector.tensor_add(g4(rpre), pg4[:, :, 0:2], gx4[:, :, 0:2])
        nc.scalar.activation(g4(r_t), g4(rpre), SIG)
        nc.vector.tensor_mul(g4(rhn), g4(r_t), pg4[:, :, 4:6])
        nc.vector.tensor_add(g4(npre), g4(rhn), gx4[:, :, 4:6])
        nc.scalar.activation(g4(n_t), g4(npre), TANH)
        nc.vector.tensor_mul(g4(nom), g4(omz), g4(n_t))
        # h' -> hR slot u%8
        nc.vector.tensor_add(hp4[:, :, :, slot], g4(nom), g4(zh))

        # -- output: layer 3 --
        if lb == L - 1:
            t3 = u - (L - 1) * SKEW
            if t3 % 32 == 0:
                ob_cur = outp.tile([HH, 32 * 2 * B], F32, tag="ob")
            ob = ob_cur
            src = hR[:].rearrange("p (l q s b) -> p l q s b", l=L, q=2, s=RING)
            nc.scalar.activation(
                ob[:, (t3 % 32) * 64:(t3 % 32 + 1) * 64]
                .rearrange("p (q b) -> p q b", q=2),
                src[:, L - 1, :, slot], TANH)
            if t3 % 32 == 31 or t3 == T - 1:
                w0 = (t3 // 32) * 32
                nw = t3 - w0 + 1
                nc.sync.dma_start(d_out[:, w0 * 64:(w0 + nw) * 64],
                                  ob[:, 0:nw * 64])

    ctx.close()


# ------------------------------------------------------------------
# host-side packing
# ------------------------------------------------------------------

def _pack_inputs(z, W_lin, b_lin, W_ih, W_hh, b_ih, b_hh, T=T_FULL):
    import ml_dtypes
    bf = ml_dtypes.bfloat16
    n_grp = T // 8

    W_linT = np.ascontiguousarray(np.asarray(W_lin).T).astype(bf)
    blin = np.ascontiguousarray(
        np.asarray(b_lin).reshape(n_grp, 8, 2, HH).transpose(0, 2, 1, 3)
        .reshape(n_grp, 16, HH)).astype(bf)
    bsel = np.zeros((16, 512), np.float32)
    for k in range(16):
        q, tp = k // 8, k % 8
        bsel[k, q * 256 + tp * B: q * 256 + (tp + 1) * B] = 1.0

    # whh[k, ((l*2+q)*6+j)*128 + m] = W_hh[l, 128j+m, 128q+k]
    def packw(Wm):
        out = np.empty((HH, L * 12 * HH), np.float32)
        Wm = np.asarray(Wm)
        for l in range(L):
            for q in range(2):
                for j in range(6):
                    c = ((l * 2 + q) * 6 + j) * HH
                    out[:, c:c + HH] = Wm[l, j * HH:(j + 1) * HH,
                                          q * HH:(q + 1) * HH].T
        return out.astype(bf)

    whh = packw(W_hh)
    wih = packw(W_ih)

    def pack6(bv):
        out = np.empty((6, L * HH), np.float32)
        bv = np.asarray(bv)
        for l in range(L):
            for j in range(6):
                out[j, l * HH:(l + 1) * HH] = bv[l, j * HH:(j + 1) * HH]
        return out.astype(bf)

    bih6 = pack6(b_ih)
    # bhhS[k, 6*lo + j, p] = b_hh[2k+lo, 128j+p]
    bhhS = np.zeros((2, 12, HH), np.float32)
    for k in range(2):
        for lo in range(2):
            for j in range(6):
                bhhS[k, 6 * lo + j] = np.asarray(b_hh)[2 * k + lo,
                                                       j * HH:(j + 1) * HH]
    # selB[6*lo + j, c] = 1 iff c in [lo*256 + 32j, lo*256 + 32(j+1))
    selB = np.zeros((12, 512), np.float32)
    for lo in range(2):
        for j in range(6):
            selB[6 * lo + j, lo * 256 + j * B: lo * 256 + (j + 1) * B] = 1.0
    selx = np.zeros((6, G3), np.float32)
    for j in range(6):
        selx[j, j * HH:(j + 1) * HH] = 1.0

    shared = dict(wlin=W_linT, blin=blin, bsel=bsel.astype(bf),
                  whh=whh, wih=wih, bih6=bih6,
                  bhhS=bhhS.astype(bf), selB=selB.astype(bf),
                  selx=selx.astype(bf))
    in_maps = []
    z = np.asarray(z)
    for c in range(N_CORES):
        zt = np.ascontiguousarray(z[c * B:(c + 1) * B, :].T).astype(bf)
        in_maps.append(dict(shared, zt=zt))
    return in_maps


def _unpack_out(raw, T=T_FULL):
    # raw [128, T*2*32] -> [32, T*256]; x[b, t, 128q+p] = raw[p, (t*2+q)*32+b]
    a = np.asarray(raw, np.float32).reshape(HH, T, 2, B)
    return np.transpose(a, (3, 1, 2, 0)).reshape(B, T * H)


_CACHED = {}


def kernel(z, W_lin, b_lin, W_ih, W_hh, b_ih, b_hh):
    import time as _time
    if "nc" not in _CACHED:
        t0 = _time.time()
        print("building program ...", flush=True)
        _CACHED["nc"] = build_program()
        print(f"build done in {_time.time() - t0:.0f}s", flush=True)
    nc = _CACHED["nc"]
    print("packing inputs ...", flush=True)
    in_maps = _pack_inputs(z, W_lin, b_lin, W_ih, W_hh, b_ih, b_hh)
    print("launching (compile on first call) ...", flush=True)
    res = run_bass_kernel_spmd(nc, in_maps, list(range(N_CORES)))
    print("run complete", flush=True)
    outs = [_unpack_out(res.results[c]["out"]) for c in range(N_CORES)]
    full = np.concatenate(outs, axis=0)
    return full.reshape(-1, H, T_FULL).astype(np.float32)

